# revision 3
# baseline (speedup 1.0000x reference)
"""Low-rank linear attention (causal, elu+1 feature map) on 8 trn2 cores.

Sharding: core = 2*b + h  (batch b in 0..3, sequence half h in 0..1).
Each core computes out[b, h*2048:(h+1)*2048, :].  Second-half cores
recompute the running K^T.V state over their 2048-token prefix on device
(prefix V contributions are scaled by sel=h so one SPMD program serves
all 8 cores).

v6 structure (from the v5 trace: PE ran the whole attention phase at
1.2 GHz because HAM re-throttled after the projection->attention lull,
and the head had ~4us of DMA-latency idle):
 - PE warm-up: ~7 dummy N=512 matmuls on a zeroed tile right after the
   preamble so HAM reaches K=8/8 before the first real matmul.
 - x DRAM layouts are consumption-ordered: prefix is chunk-major
   ([128 dd, ci*1024 + d*128 + t]), main is group-major
   ([128 dd, g*4096 + d*512 + t]); transfers are interleaved so main
   group 0 lands before the last prefix chunks are consumed.
 - main projections are interleaved with phase-B chunks: each phase-B
   chunk is preceded by the [kf|v] token-major chunk projection for
   chunk ci+4 (8 MMs N=128, same wkv weights as the prefix  -> no PE
   transposes, no N=64 V matmuls) and, at group boundaries, the 8-MM
   N=512 K-major [q;k] group projection.  The PE stream stays dense so
   HAM never re-throttles and vector-latency waits are absorbed.
 - output is chunk-major in DRAM ([128 t, ci*1024 + dcol]); evictions
   land in one persistent SBUF tile split scalar/vector per half, and
   stores are 3 big 1MB group transfers + a 768KB + 2x128KB tail.
   The host reassembles [16,128,1024] -> [2048, 1024] for free.
 - elu+1 = max(z+1, min(exp(z), 1)) on scalar+vector; V carries 1/16.
 - PSUM budget: state(1) + p1(1) + pp2(1) + att(3) + op(2) = 8 banks.

Shapes (hardcoded): B=4, S=4096, D=1024, K=64.  L = S/2 = 2048 tokens
per core, processed in 16 chunks of C=128.
"""

import numpy as np

B, S, D, K = 4, 4096, 1024, 64
L = S // 2          # tokens per core (main), also prefix length
C = 128             # chunk (tokens)
G = 512             # token group for K-major projection matmuls
NCHUNK = L // C     # 16
NGRP = L // G       # 4
NDC = D // 128      # 8 contraction chunks
VS = 1.0 / 16.0     # V scale folded into num and den

_cache = {}


def _build_nc():
    import concourse.bacc as bacc
    import concourse.tile as tile
    from concourse import mybir

    f32 = mybir.dt.float32
    bf16 = mybir.dt.bfloat16
    AF = mybir.ActivationFunctionType
    Alu = mybir.AluOpType

    nc = bacc.Bacc()

    # chunk-major prefix x: [dd, ci*1024 + d*128 + t]
    xtp2 = nc.declare_dram_parameter("xtp2", [128, NCHUNK * 1024], bf16,
                                     isOutput=False)
    # group-major main x: [dd, g*4096 + d*512 + t]
    xtm2 = nc.declare_dram_parameter("xtm2", [128, NGRP * 4096], bf16,
                                     isOutput=False)
    # wcat cols: [wkv 8*128 | wqk 8*128 | mask 128]
    WKV0, WQK0 = 0, NDC * 128
    MSK0 = 2 * NDC * 128
    WCOLS = MSK0 + C
    wcat = nc.declare_dram_parameter("wcat", [128, WCOLS], bf16,
                                     isOutput=False)
    wot = nc.declare_dram_parameter("wot", [K, D], bf16, isOutput=False)
    selc = nc.declare_dram_parameter("selc", [C, 1], f32, isOutput=False)
    # chunk-major output: [t, ci*1024 + dcol]
    out2 = nc.declare_dram_parameter("out2", [128, NCHUNK * 1024], bf16,
                                     isOutput=True)

    with tile.TileContext(nc) as tc:
        with (
            tc.tile_pool(name="consts", bufs=1) as consts,
            tc.tile_pool(name="xp", bufs=1) as xp_pool,
            tc.tile_pool(name="xg", bufs=1) as xg_pool,
            tc.tile_pool(name="ost", bufs=1) as ost_pool,
            tc.tile_pool(name="small", bufs=4) as small,
            tc.tile_pool(name="vko", bufs=NCHUNK + 1) as vko_pool,
            tc.tile_pool(name="pvko", bufs=NCHUNK + 1) as pvko_pool,
            tc.tile_pool(name="qk", bufs=NCHUNK + 1) as qk_pool,
            tc.tile_pool(name="atn", bufs=4) as atn_pool,
            tc.tile_pool(name="ks", bufs=4) as ks_pool,
            tc.tile_pool(name="tmp", bufs=4) as tmp_pool,
            tc.tile_pool(name="ptmp", bufs=4) as ptmp_pool,
            tc.tile_pool(name="state_pool", bufs=1, space="PSUM") as state_pool,
        ):
            # ---- weights/consts on the scalar HWDGE queue (selc + wkv
            # first: the first prefix matmul + evict need them) ----
            selc_sb = consts.tile([C, 1], f32, tag="selc")
            nc.scalar.dma_start(out=selc_sb, in_=selc[:, :])
            wcat_sb = consts.tile([128, WCOLS], bf16, tag="wcat")
            nc.scalar.dma_start(out=wcat_sb[:, WKV0:WKV0 + NDC * 128],
                                in_=wcat[:, WKV0:WKV0 + NDC * 128])
            nc.scalar.dma_start(out=wcat_sb[:, WQK0:WCOLS],
                                in_=wcat[:, WQK0:WCOLS])
            wot_sb = consts.tile([K, D], bf16, tag="wot")
            nc.scalar.dma_start(out=wot_sb, in_=wot[:, :])
            wkv_sb = [wcat_sb[:, WKV0 + d * 128:WKV0 + (d + 1) * 128]
                      for d in range(NDC)]
            wqk_sb = [wcat_sb[:, WQK0 + d * 128:WQK0 + (d + 1) * 128]
                      for d in range(NDC)]
            mask_sb = wcat_sb[:, MSK0:MSK0 + C]

            # ---- x transfers on the sync HWDGE queue, consumption
            # order: prefix chunks 0..11, main group0 half, prefix
            # 12..15, rest of main (d-halves so qk matmuls can start on
            # the first half) ----
            xpc = [xp_pool.tile([128, 1024], bf16, name=f"xpc{ci}",
                                tag=f"xpc{ci}") for ci in range(NCHUNK)]
            xg = [xg_pool.tile([128, 4096], bf16, name=f"xg{g}",
                               tag=f"xg{g}") for g in range(NGRP)]

            def dma_xg_half(g, half):
                lo = g * 4096 + half * 2048
                nc.sync.dma_start(out=xg[g][:, half * 2048:(half + 1) * 2048],
                                  in_=xtm2[:, lo:lo + 2048])

            for ci in range(12):
                nc.sync.dma_start(out=xpc[ci],
                                  in_=xtp2[:, ci * 1024:(ci + 1) * 1024])
            dma_xg_half(0, 0)
            for ci in range(12, NCHUNK):
                nc.sync.dma_start(out=xpc[ci],
                                  in_=xtp2[:, ci * 1024:(ci + 1) * 1024])
            dma_xg_half(0, 1)
            for g in range(1, NGRP):
                dma_xg_half(g, 0)
                dma_xg_half(g, 1)

            # den-sum column: carries the same 1/16 scale as V's columns
            onec_sb = consts.tile([C, 1], bf16, tag="onec")
            nc.vector.memset(onec_sb, VS)
            # persistent output staging (chunk-major, matches out2)
            ostage = ost_pool.tile([128, NCHUNK * 1024], bf16, tag="ostage")

            # running state [K, K+1]: cols 0:K = S'[k,m] (1/16-scaled),
            # col K = k_sum/16.
            state_ps = state_pool.tile([K, 1 + K], f32)

            # =============== PE warm-up: dummy matmuls on zeros =======
            wz = consts.tile([128, G], bf16, tag="wz")
            nc.vector.memset(wz, 0.0)

            # =============== PREFIX: token-major [K|V], state sum ======
            pvkos = []
            with tc.tile_pool(name="pp_ps", bufs=4, space="PSUM") as pp_pool:
                with tc.tile_pool(name="warm_ps", bufs=1,
                                  space="PSUM") as warm_pool:
                    warm_ps = warm_pool.tile([128, G], f32)
                    for i in range(7):
                        nc.tensor.matmul(warm_ps, wz[:, 0:128], wz,
                                         start=True, stop=True)
                for ci in range(NCHUNK):
                    pp = pp_pool.tile([C, 2 * K], f32, tag="pp")
                    for d in range(NDC):
                        nc.tensor.matmul(pp,
                                         xpc[ci][:, d * 128:(d + 1) * 128],
                                         wkv_sb[d],
                                         start=(d == 0), stop=(d == NDC - 1))
                    eu = ptmp_pool.tile([C, K], f32, tag="eu")
                    nc.scalar.activation(eu, pp[:, 0:K], AF.Exp)
                    em = ptmp_pool.tile([C, K], f32, tag="em")
                    nc.vector.tensor_scalar_min(em, eu, 1.0)
                    pvko = pvko_pool.tile([C, 2 * K + 1], bf16, tag="pvko")
                    nc.vector.scalar_tensor_tensor(
                        pvko[:, 0:K], pp[:, 0:K], 1.0, em,
                        op0=Alu.add, op1=Alu.max)
                    # V and ones column scaled by sel/16 (zero on h=0)
                    nc.scalar.activation(pvko[:, K:2 * K], pp[:, K:2 * K],
                                         AF.Copy, scale=selc_sb)
                    nc.gpsimd.tensor_copy(pvko[:, 2 * K:2 * K + 1], selc_sb)
                    pvkos.append(pvko)
                # state updates after all projections: PE never stalls
                for ci in range(NCHUNK):
                    pvko = pvkos[ci]
                    nc.tensor.matmul(state_ps, pvko[:, 0:K],
                                     pvko[:, K:2 * K + 1],
                                     start=(ci == 0), stop=False,
                                     skip_group_check=True)

            # prefix state snapshot (ks for chunk 0)
            ks_init = ks_pool.tile([K, 1 + K], bf16, tag="ks")
            nc.scalar.copy(ks_init, state_ps)

            # =============== MAIN: projections interleaved with =======
            # =============== phase-B attention chunks          =======
            qTs, kTs, vkos = {}, {}, {}
            p1gs, eg2s = {}, {}
            atms = {}
            with (
                tc.tile_pool(name="p1_ps", bufs=1, space="PSUM") as p1_pool,
                tc.tile_pool(name="pp2_ps", bufs=1, space="PSUM") as pp2_pool,
                tc.tile_pool(name="att_ps", bufs=3, space="PSUM") as att_pool,
                tc.tile_pool(name="op_ps", bufs=1, space="PSUM") as op_pool,
            ):
                def emit_qk(g):
                    """K-major [q;k] group projection (8 MMs N=512)."""
                    p1g = p1_pool.tile([2 * K, G], f32, tag="p1")
                    for d in range(NDC):
                        nc.tensor.matmul(p1g, wqk_sb[d],
                                         xg[g][:, d * 512:(d + 1) * 512],
                                         start=(d == 0), stop=(d == NDC - 1))
                    eg = tmp_pool.tile([2 * K, G], f32, tag="eg", bufs=2)
                    nc.scalar.activation(eg, p1g, AF.Exp)
                    eg2 = tmp_pool.tile([2 * K, G], f32, tag="eg2", bufs=2)
                    nc.vector.tensor_scalar_min(eg2, eg, 1.0)
                    p1gs[g], eg2s[g] = p1g, eg2

                def emit_qkT(ci):
                    """qT/kT (K-major, elu'd) for chunk ci from its group
                    projection."""
                    g, c4 = ci // 4, ci % 4
                    p1g, eg2 = p1gs[g], eg2s[g]
                    sl = slice(c4 * C, (c4 + 1) * C)
                    kT = qk_pool.tile([K, C], bf16, tag="kT")
                    nc.vector.scalar_tensor_tensor(
                        kT, p1g[K:2 * K, sl], 1.0, eg2[K:2 * K, sl],
                        op0=Alu.add, op1=Alu.max)
                    qT = qk_pool.tile([K, C], bf16, tag="qT")
                    nc.vector.scalar_tensor_tensor(
                        qT, p1g[0:K, sl], 1.0, eg2[0:K, sl],
                        op0=Alu.add, op1=Alu.max)
                    qTs[ci], kTs[ci] = qT, kT

                def emit_kv(ci):
                    """token-major [kf|v|1] for chunk ci (8 MMs N=128,
                    same wkv weights as the prefix)."""
                    g, c4 = ci // 4, ci % 4
                    pp2 = pp2_pool.tile([C, 2 * K], f32, tag="pp2")
                    for d in range(NDC):
                        lo = d * 512 + c4 * 128
                        nc.tensor.matmul(pp2, xg[g][:, lo:lo + 128],
                                         wkv_sb[d],
                                         start=(d == 0), stop=(d == NDC - 1))
                    eu2 = ptmp_pool.tile([C, K], f32, tag="eu2")
                    nc.scalar.activation(eu2, pp2[:, 0:K], AF.Exp)
                    em2 = ptmp_pool.tile([C, K], f32, tag="em2")
                    nc.vector.tensor_scalar_min(em2, eu2, 1.0)
                    vko = vko_pool.tile([C, 2 * K + 1], bf16, tag="vko")
                    nc.vector.scalar_tensor_tensor(
                        vko[:, 0:K], pp2[:, 0:K], 1.0, em2,
                        op0=Alu.add, op1=Alu.max)
                    nc.scalar.activation(vko[:, K:2 * K], pp2[:, K:2 * K],
                                         AF.Copy, scale=VS)
                    nc.gpsimd.memset(vko[:, 2 * K:2 * K + 1], VS)
                    vkos[ci] = vko

                def scores(ci):
                    at = att_pool.tile([C, C], f32, tag="att")
                    nc.tensor.matmul(at, kTs[ci], qTs[ci], start=True,
                                     stop=True)
                    atm = atn_pool.tile([C, C], bf16, tag="atm")
                    nc.vector.tensor_tensor(atm, at, mask_sb, Alu.mult)
                    atms[ci] = atm

                def outproj(ci, attn, recip):
                    op = op_pool.tile([C, D], f32, tag="op")
                    nc.tensor.matmul(op[:, 0:512], attn, wot_sb[:, 0:512],
                                     start=True, stop=True)
                    nc.tensor.matmul(op[:, 512:1024], attn,
                                     wot_sb[:, 512:1024],
                                     start=True, stop=True)
                    lo = ci * 1024
                    nc.scalar.activation(ostage[:, lo:lo + 512],
                                         op[:, 0:512], AF.Copy, scale=recip)
                    nc.vector.tensor_scalar_mul(ostage[:, lo + 512:lo + 1024],
                                                op[:, 512:1024], recip)

                # prologue: group 0 + its four chunk projections, then
                # the first two score lookaheads
                emit_qk(0)
                for ci in range(4):
                    emit_qkT(ci)
                    emit_kv(ci)
                scores(0)
                scores(1)

                ks_prev = ks_init
                prev = None  # (ci, attn, recip) pending output projection
                for ci in range(NCHUNK):
                    g, c4 = ci // 4, ci % 4
                    # projection filler for later chunks (keeps PE dense
                    # and absorbs the state-chain vector latency)
                    if c4 == 0 and g + 1 < NGRP:
                        emit_qk(g + 1)
                    if ci + 4 < NCHUNK:
                        emit_qkT(ci + 4)
                        emit_kv(ci + 4)
                    if prev is not None:
                        outproj(*prev)
                        pci = prev[0]
                        if pci == 3:
                            nc.sync.dma_start(out=out2[:, 0:4096],
                                              in_=ostage[:, 0:4096])
                        elif pci == 7:
                            nc.sync.dma_start(out=out2[:, 4096:8192],
                                              in_=ostage[:, 4096:8192])
                        elif pci == 11:
                            nc.sync.dma_start(out=out2[:, 8192:12288],
                                              in_=ostage[:, 8192:12288])
                        elif pci == 14:
                            nc.sync.dma_start(out=out2[:, 12288:15360],
                                              in_=ostage[:, 12288:15360])
                    qT, vko, atm = qTs[ci], vkos[ci], atms.pop(ci)
                    # [num'^T] = v'^T A + S'^T q
                    nd = att_pool.tile([K, C], f32, tag="att")
                    nc.tensor.matmul(nd, vko[:, K:2 * K], atm,
                                     start=True, stop=False)
                    nc.tensor.matmul(nd, ks_prev[:, 0:K], qT,
                                     start=False, stop=True)
                    # den' transposed directly on the PE: [C,1]; recip
                    # reads PSUM — no vector hop inside the PE chain
                    den = att_pool.tile([C, 1], f32, tag="att")
                    nc.tensor.matmul(den, atm, onec_sb, start=True,
                                     stop=False)
                    nc.tensor.matmul(den, qT, ks_prev[:, K:K + 1],
                                     start=False, stop=True)
                    # state update + snapshot
                    nc.tensor.matmul(state_ps, vko[:, 0:K],
                                     vko[:, K:2 * K + 1],
                                     start=False, stop=(ci == NCHUNK - 1),
                                     skip_group_check=True)
                    if ci + 2 < NCHUNK:
                        scores(ci + 2)
                    ks_i = ks_pool.tile([K, 1 + K], bf16, tag="ks")
                    nc.vector.tensor_copy(ks_i, state_ps)
                    ks_prev = ks_i
                    recip = small.tile([C, 1], f32, tag="recip")
                    nc.vector.reciprocal(recip, den)
                    attn = atn_pool.tile([K, C], bf16, tag="attn")
                    nc.vector.tensor_copy(attn, nd)
                    prev = (ci, attn, recip)
                outproj(*prev)
                # tail: last chunk's store split across the two HWDGE
                # queues so it doesn't serialize the finish
                nc.sync.dma_start(out=out2[:, 15360:15872],
                                  in_=ostage[:, 15360:15872])
                nc.scalar.dma_start(out=out2[:, 15872:16384],
                                    in_=ostage[:, 15872:16384])

    nc.compile()
    worst = []
    for fn in nc.m.functions:
        for blk in fn.blocks:
            for inst in blk.instructions:
                n = len(inst.sync_info.on_wait) if inst.sync_info else 0
                if n > 1 and type(inst).__name__ == "InstMatmult":
                    worst.append((inst.name, n))
    if worst:
        print(f"WARNING: matmuls with >1 wait after lowering: {worst}")
    return nc


def _prep_inputs(x, Wq, Wk, Wv, Wo):
    import ml_dtypes

    bf16 = ml_dtypes.bfloat16
    wkv = np.concatenate([Wk.T, Wv.T], axis=1)                # [D, 2K]
    wqk = np.concatenate([Wq.T, Wk.T], axis=1)                # [D, 2K]
    mask = np.triu(np.ones((C, C), np.float32))               # keep t <= s
    wcat = np.concatenate(
        [wkv[d * 128:(d + 1) * 128, :] for d in range(NDC)]
        + [wqk[d * 128:(d + 1) * 128, :] for d in range(NDC)]
        + [mask],
        axis=1,
    ).astype(bf16)
    wot = np.ascontiguousarray(Wo.T).astype(bf16)             # [K, D]
    zeros_xp = np.zeros((128, NCHUNK * 1024), dtype=bf16)
    in_maps = []
    for core in range(8):
        b, h = core // 2, core % 2
        xb = x[b].astype(bf16)                                # [S, D]
        # main: [dd, g*4096 + d*512 + t]
        xm = xb[h * L:(h + 1) * L, :]                         # [2048, 1024]
        xm4 = xm.reshape(NGRP, G, NDC, 128).transpose(3, 0, 2, 1)
        xtm2 = np.ascontiguousarray(xm4.reshape(128, NGRP * 4096))
        # prefix: [dd, ci*1024 + d*128 + t]
        if h:
            xp = xb[0:L, :]
            xp4 = xp.reshape(NCHUNK, C, NDC, 128).transpose(3, 0, 2, 1)
            xtp2 = np.ascontiguousarray(xp4.reshape(128, NCHUNK * 1024))
        else:
            xtp2 = zeros_xp
        m = {
            "xtp2": xtp2,
            "xtm2": xtm2,
            "wcat": wcat,
            "wot": wot,
            "selc": np.full((C, 1), float(h) / 16.0, np.float32),
        }
        in_maps.append(m)
    return in_maps


def _run(inputs, trace=False):
    from concourse.bass_utils import run_bass_kernel_spmd

    if "nc" not in _cache:
        _cache["nc"] = _build_nc()
    nc = _cache["nc"]
    in_maps = _prep_inputs(
        np.asarray(inputs["x"], np.float32),
        np.asarray(inputs["Wq"], np.float32),
        np.asarray(inputs["Wk"], np.float32),
        np.asarray(inputs["Wv"], np.float32),
        np.asarray(inputs["Wo"], np.float32),
    )
    res = run_bass_kernel_spmd(nc, in_maps, list(range(8)), trace=trace)
    out = np.empty((B, S, D), np.float32)
    for core in range(8):
        b, h = core // 2, core % 2
        o = res.results[core]["out2"].astype(np.float32)
        # [128 t, ci*1024 + dcol] -> [2048, 1024]
        o = o.reshape(128, NCHUNK, 1024).transpose(1, 0, 2).reshape(L, D)
        out[b, h * L:(h + 1) * L, :] = o
    return out, res


def kernel(**inputs) -> np.ndarray:
    out, _ = _run(inputs, trace=False)
    return out


# revision 4
# speedup vs baseline: 1.1282x; 1.1282x over previous
"""Low-rank linear attention (causal, elu+1 feature map) on 8 trn2 cores.

Sharding: core = 2*b + h  (batch b in 0..3, sequence half h in 0..1).
Each core computes out[b, h*2048:(h+1)*2048, :].  Second-half cores
recompute the running K^T.V state over their 2048-token prefix on device
(prefix V contributions are scaled by sel=h so one SPMD program serves
all 8 cores).

v7 structure (v5 ran the whole attention phase at 1.2 GHz because HAM
re-throttled in the projection->attention lull; v6's interleaving
head-of-line-blocked the in-order PE on DMA waits):
 - PE warm-up: ~11 dummy N=512 matmuls on a zeroed tile right after the
   preamble so HAM latches K=8/8 before the first real matmul and the
   PE never sees a >3.4us idle window at the head.
 - x DRAM layouts are consumption-ordered: prefix chunk-major
   ([128 dd, ci*1024 + d*128 + t]), main group-major
   ([128 dd, g*4096 + d*512 + t]).
 - main projections: K-major [q;k] group matmuls (N=512) + token-major
   [kf|v] chunk matmuls (N=128, same wkv weights as the prefix) -> no
   PE transposes, no N=64 V matmuls.
 - phase B processes chunk PAIRS: per pair one extra unmasked cross
   score block A01 lets both chunks' num/den run off the pair-boundary
   state snapshot, halving the serial state->snapshot->num round trips
   and making each pair a dense ~19-matmul PE burst (HAM stays warm).
   Snapshot copy runs on scalar so the clogged vector queue is off the
   serial chain; evictions are split scalar/vector per half-chunk.
 - output is chunk-major in DRAM; evictions land in one persistent SBUF
   tile; stores are 3x 1MB group transfers + 768KB + 2x128KB tail.
 - elu+1 = max(z+1, min(exp(z), 1)); V carries 1/16 (num/den scale).
 - PSUM: phase A state(1)+p1(2)+pp2(2) <= 8; phase B state(1)+att(4)+
   op(3x [C,512]) = 8 banks.

Shapes (hardcoded): B=4, S=4096, D=1024, K=64.  L = S/2 = 2048 tokens
per core, processed in 16 chunks of C=128 (8 pairs).
"""

import numpy as np

B, S, D, K = 4, 4096, 1024, 64
L = S // 2          # tokens per core (main), also prefix length
C = 128             # chunk (tokens)
G = 512             # token group for K-major projection matmuls
NCHUNK = L // C     # 16
NPAIR = NCHUNK // 2  # 8
NGRP = L // G       # 4
NDC = D // 128      # 8 contraction chunks
VS = 1.0 / 16.0     # V scale folded into num and den

_cache = {}


def _build_nc():
    import concourse.bacc as bacc
    import concourse.tile as tile
    from concourse import mybir

    f32 = mybir.dt.float32
    bf16 = mybir.dt.bfloat16
    AF = mybir.ActivationFunctionType
    Alu = mybir.AluOpType

    nc = bacc.Bacc()

    # chunk-major prefix x: [dd, ci*1024 + d*128 + t]
    xtp2 = nc.declare_dram_parameter("xtp2", [128, NCHUNK * 1024], bf16,
                                     isOutput=False)
    # group-major main x: [dd, g*4096 + d*512 + t]
    xtm2 = nc.declare_dram_parameter("xtm2", [128, NGRP * 4096], bf16,
                                     isOutput=False)
    # wcat cols: [wkv 8*128 | wqk 8*128 | mask 128]
    WKV0, WQK0 = 0, NDC * 128
    MSK0 = 2 * NDC * 128
    WCOLS = MSK0 + C
    wcat = nc.declare_dram_parameter("wcat", [128, WCOLS], bf16,
                                     isOutput=False)
    wot = nc.declare_dram_parameter("wot", [K, D], bf16, isOutput=False)
    selc = nc.declare_dram_parameter("selc", [C, 1], f32, isOutput=False)
    # chunk-major output: [t, ci*1024 + dcol]
    out2 = nc.declare_dram_parameter("out2", [128, NCHUNK * 1024], bf16,
                                     isOutput=True)

    with tile.TileContext(nc) as tc:
        with (
            tc.tile_pool(name="consts", bufs=1) as consts,
            tc.tile_pool(name="xp", bufs=1) as xp_pool,
            tc.tile_pool(name="xg", bufs=1) as xg_pool,
            tc.tile_pool(name="ost", bufs=1) as ost_pool,
            tc.tile_pool(name="small", bufs=6) as small,
            tc.tile_pool(name="vko", bufs=NCHUNK + 1) as vko_pool,
            tc.tile_pool(name="pvko", bufs=NCHUNK + 1) as pvko_pool,
            tc.tile_pool(name="qk", bufs=NCHUNK + 1) as qk_pool,
            tc.tile_pool(name="atn", bufs=8) as atn_pool,
            tc.tile_pool(name="ks", bufs=4) as ks_pool,
            tc.tile_pool(name="tmp", bufs=4) as tmp_pool,
            tc.tile_pool(name="ptmp", bufs=4) as ptmp_pool,
            tc.tile_pool(name="state_pool", bufs=1, space="PSUM") as state_pool,
        ):
            # ---- weights/consts on the scalar HWDGE queue (selc + wkv
            # first: the first prefix matmul + evict need them) ----
            selc_sb = consts.tile([C, 1], f32, tag="selc")
            nc.scalar.dma_start(out=selc_sb, in_=selc[:, :])
            wcat_sb = consts.tile([128, WCOLS], bf16, tag="wcat")
            nc.scalar.dma_start(out=wcat_sb[:, WKV0:WKV0 + NDC * 128],
                                in_=wcat[:, WKV0:WKV0 + NDC * 128])
            nc.scalar.dma_start(out=wcat_sb[:, WQK0:WCOLS],
                                in_=wcat[:, WQK0:WCOLS])
            wot_sb = consts.tile([K, D], bf16, tag="wot")
            nc.scalar.dma_start(out=wot_sb, in_=wot[:, :])
            wkv_sb = [wcat_sb[:, WKV0 + d * 128:WKV0 + (d + 1) * 128]
                      for d in range(NDC)]
            wqk_sb = [wcat_sb[:, WQK0 + d * 128:WQK0 + (d + 1) * 128]
                      for d in range(NDC)]
            mask_sb = wcat_sb[:, MSK0:MSK0 + C]

            # ---- x transfers on the sync HWDGE queue, consumption
            # order: prefix chunks then main groups (d-halves so qk
            # matmuls can start on the first half) ----
            xpc = [xp_pool.tile([128, 1024], bf16, name=f"xpc{ci}",
                                tag=f"xpc{ci}") for ci in range(NCHUNK)]
            xg = [xg_pool.tile([128, 4096], bf16, name=f"xg{g}",
                               tag=f"xg{g}") for g in range(NGRP)]
            for ci in range(NCHUNK):
                nc.sync.dma_start(out=xpc[ci],
                                  in_=xtp2[:, ci * 1024:(ci + 1) * 1024])
            for g in range(NGRP):
                for half in range(2):
                    lo = g * 4096 + half * 2048
                    nc.sync.dma_start(
                        out=xg[g][:, half * 2048:(half + 1) * 2048],
                        in_=xtm2[:, lo:lo + 2048])

            # den-sum column: carries the same 1/16 scale as V's columns
            onec_sb = consts.tile([C, 1], bf16, tag="onec")
            nc.vector.memset(onec_sb, VS)
            # persistent output staging (chunk-major, matches out2)
            ostage = ost_pool.tile([128, NCHUNK * 1024], bf16, tag="ostage")

            # running state [K, K+1]: cols 0:K = S'[k,m] (1/16-scaled),
            # col K = k_sum/16.
            state_ps = state_pool.tile([K, 1 + K], f32)

            # =============== PE warm-up: dummy matmuls on zeros =======
            wz = consts.tile([128, G], bf16, tag="wz")
            nc.vector.memset(wz, 0.0)

            # =============== PREFIX: token-major [K|V], state sum ======
            pvkos = []
            with tc.tile_pool(name="pp_ps", bufs=4, space="PSUM") as pp_pool:
                with tc.tile_pool(name="warm_ps", bufs=1,
                                  space="PSUM") as warm_pool:
                    warm_ps = warm_pool.tile([128, G], f32)
                    for i in range(11):
                        nc.tensor.matmul(warm_ps, wz[:, 0:128], wz,
                                         start=True, stop=True)
                for ci in range(NCHUNK):
                    pp = pp_pool.tile([C, 2 * K], f32, tag="pp")
                    for d in range(NDC):
                        nc.tensor.matmul(pp,
                                         xpc[ci][:, d * 128:(d + 1) * 128],
                                         wkv_sb[d],
                                         start=(d == 0), stop=(d == NDC - 1))
                    eu = ptmp_pool.tile([C, K], f32, tag="eu")
                    nc.scalar.activation(eu, pp[:, 0:K], AF.Exp)
                    em = ptmp_pool.tile([C, K], f32, tag="em")
                    nc.vector.tensor_scalar_min(em, eu, 1.0)
                    pvko = pvko_pool.tile([C, 2 * K + 1], bf16, tag="pvko")
                    nc.vector.scalar_tensor_tensor(
                        pvko[:, 0:K], pp[:, 0:K], 1.0, em,
                        op0=Alu.add, op1=Alu.max)
                    # V and ones column scaled by sel/16 (zero on h=0)
                    nc.scalar.activation(pvko[:, K:2 * K], pp[:, K:2 * K],
                                         AF.Copy, scale=selc_sb)
                    nc.gpsimd.tensor_copy(pvko[:, 2 * K:2 * K + 1], selc_sb)
                    pvkos.append(pvko)
                # state updates after all projections: PE never stalls
                for ci in range(NCHUNK):
                    pvko = pvkos[ci]
                    nc.tensor.matmul(state_ps, pvko[:, 0:K],
                                     pvko[:, K:2 * K + 1],
                                     start=(ci == 0), stop=False,
                                     skip_group_check=True)

            # prefix state snapshot (ks for pair 0)
            ks_init = ks_pool.tile([K, 1 + K], bf16, tag="ks")
            nc.scalar.copy(ks_init, state_ps)

            # =============== MAIN projections =========================
            qTs, kTs, vkos = {}, {}, {}
            with (
                tc.tile_pool(name="p1_ps", bufs=2, space="PSUM") as p1_pool,
                tc.tile_pool(name="pp2_ps", bufs=2, space="PSUM") as pp2_pool,
            ):
                for g in range(NGRP):
                    p1g = p1_pool.tile([2 * K, G], f32, tag="p1")
                    for d in range(NDC):
                        nc.tensor.matmul(p1g, wqk_sb[d],
                                         xg[g][:, d * 512:(d + 1) * 512],
                                         start=(d == 0), stop=(d == NDC - 1))
                    eg = tmp_pool.tile([2 * K, G], f32, tag="eg", bufs=2)
                    nc.scalar.activation(eg, p1g, AF.Exp)
                    eg2 = tmp_pool.tile([2 * K, G], f32, tag="eg2", bufs=2)
                    nc.vector.tensor_scalar_min(eg2, eg, 1.0)
                    for c4 in range(4):
                        ci = g * 4 + c4
                        sl = slice(c4 * C, (c4 + 1) * C)
                        kT = qk_pool.tile([K, C], bf16, tag="kT")
                        nc.vector.scalar_tensor_tensor(
                            kT, p1g[K:2 * K, sl], 1.0, eg2[K:2 * K, sl],
                            op0=Alu.add, op1=Alu.max)
                        qT = qk_pool.tile([K, C], bf16, tag="qT")
                        nc.vector.scalar_tensor_tensor(
                            qT, p1g[0:K, sl], 1.0, eg2[0:K, sl],
                            op0=Alu.add, op1=Alu.max)
                        qTs[ci], kTs[ci] = qT, kT
                        # token-major [kf|v|1] (same wkv as the prefix)
                        pp2 = pp2_pool.tile([C, 2 * K], f32, tag="pp2")
                        for d in range(NDC):
                            lo = d * 512 + c4 * 128
                            nc.tensor.matmul(pp2, xg[g][:, lo:lo + 128],
                                             wkv_sb[d],
                                             start=(d == 0),
                                             stop=(d == NDC - 1))
                        eu2 = ptmp_pool.tile([C, K], f32, tag="eu2")
                        nc.scalar.activation(eu2, pp2[:, 0:K], AF.Exp)
                        em2 = ptmp_pool.tile([C, K], f32, tag="em2")
                        nc.vector.tensor_scalar_min(em2, eu2, 1.0)
                        vko = vko_pool.tile([C, 2 * K + 1], bf16, tag="vko")
                        nc.vector.scalar_tensor_tensor(
                            vko[:, 0:K], pp2[:, 0:K], 1.0, em2,
                            op0=Alu.add, op1=Alu.max)
                        nc.scalar.activation(vko[:, K:2 * K],
                                             pp2[:, K:2 * K],
                                             AF.Copy, scale=VS)
                        nc.gpsimd.memset(vko[:, 2 * K:2 * K + 1], VS)
                        vkos[ci] = vko

            # =============== PHASE B: attention (chunk pairs) =========
            with (
                tc.tile_pool(name="att_ps", bufs=4, space="PSUM") as att_pool,
                tc.tile_pool(name="op_ps", bufs=3, space="PSUM") as op_pool,
            ):
                atms = {}   # (pair, kind): kind 0=A00, 1=A01, 2=A11

                def ablocks(p):
                    a, b = 2 * p, 2 * p + 1
                    at0 = att_pool.tile([C, C], f32, tag="att")
                    nc.tensor.matmul(at0, kTs[a], qTs[a], start=True,
                                     stop=True)
                    m0 = atn_pool.tile([C, C], bf16, tag="atm")
                    nc.vector.tensor_tensor(m0, at0, mask_sb, Alu.mult)
                    at1 = att_pool.tile([C, C], f32, tag="att")
                    nc.tensor.matmul(at1, kTs[a], qTs[b], start=True,
                                     stop=True)
                    m1 = atn_pool.tile([C, C], bf16, tag="atm")
                    nc.scalar.copy(m1, at1)  # unmasked cross block
                    at2 = att_pool.tile([C, C], f32, tag="att")
                    nc.tensor.matmul(at2, kTs[b], qTs[b], start=True,
                                     stop=True)
                    m2 = atn_pool.tile([C, C], bf16, tag="atm")
                    nc.vector.tensor_tensor(m2, at2, mask_sb, Alu.mult)
                    atms[(p, 0)], atms[(p, 1)], atms[(p, 2)] = m0, m1, m2

                def outproj(ci, attn, recip):
                    op1 = op_pool.tile([C, 512], f32, tag="op")
                    op2 = op_pool.tile([C, 512], f32, tag="op")
                    nc.tensor.matmul(op1, attn, wot_sb[:, 0:512],
                                     start=True, stop=True)
                    nc.tensor.matmul(op2, attn, wot_sb[:, 512:1024],
                                     start=True, stop=True)
                    lo = ci * 1024
                    # split the eviction across scalar+vector
                    nc.scalar.activation(ostage[:, lo:lo + 512], op1,
                                         AF.Copy, scale=recip)
                    nc.vector.tensor_scalar_mul(
                        ostage[:, lo + 512:lo + 1024], op2, recip)
                    if ci == 3:
                        nc.sync.dma_start(out=out2[:, 0:4096],
                                          in_=ostage[:, 0:4096])
                    elif ci == 7:
                        nc.sync.dma_start(out=out2[:, 4096:8192],
                                          in_=ostage[:, 4096:8192])
                    elif ci == 11:
                        nc.sync.dma_start(out=out2[:, 8192:12288],
                                          in_=ostage[:, 8192:12288])
                    elif ci == 14:
                        nc.sync.dma_start(out=out2[:, 12288:15360],
                                          in_=ostage[:, 12288:15360])

                ablocks(0)
                ks_prev = ks_init
                prevs = []  # pending (ci, attn, recip) output projections
                for p in range(NPAIR):
                    a, b = 2 * p, 2 * p + 1
                    for pr in prevs:
                        outproj(*pr)
                    prevs = []
                    m0 = atms.pop((p, 0))
                    m1 = atms.pop((p, 1))
                    m2 = atms.pop((p, 2))
                    qTa, qTb = qTs[a], qTs[b]
                    vka, vkb = vkos[a], vkos[b]
                    # chunk a: num/den off the pair-boundary snapshot
                    nda = att_pool.tile([K, C], f32, tag="att")
                    nc.tensor.matmul(nda, vka[:, K:2 * K], m0,
                                     start=True, stop=False)
                    nc.tensor.matmul(nda, ks_prev[:, 0:K], qTa,
                                     start=False, stop=True)
                    dena = att_pool.tile([C, 1], f32, tag="att")
                    nc.tensor.matmul(dena, m0, onec_sb, start=True,
                                     stop=False)
                    nc.tensor.matmul(dena, qTa, ks_prev[:, K:K + 1],
                                     start=False, stop=True)
                    recipa = small.tile([C, 1], f32, tag="recip")
                    nc.vector.reciprocal(recipa, dena)
                    attna = atn_pool.tile([K, C], bf16, tag="attn")
                    nc.vector.tensor_copy(attna, nda)
                    # chunk b: adds the cross-block contribution
                    ndb = att_pool.tile([K, C], f32, tag="att")
                    nc.tensor.matmul(ndb, vkb[:, K:2 * K], m2,
                                     start=True, stop=False)
                    nc.tensor.matmul(ndb, vka[:, K:2 * K], m1,
                                     start=False, stop=False)
                    nc.tensor.matmul(ndb, ks_prev[:, 0:K], qTb,
                                     start=False, stop=True)
                    denb = att_pool.tile([C, 1], f32, tag="att")
                    nc.tensor.matmul(denb, m2, onec_sb, start=True,
                                     stop=False)
                    nc.tensor.matmul(denb, m1, onec_sb, start=False,
                                     stop=False)
                    nc.tensor.matmul(denb, qTb, ks_prev[:, K:K + 1],
                                     start=False, stop=True)
                    recipb = small.tile([C, 1], f32, tag="recip")
                    nc.vector.reciprocal(recipb, denb)
                    attnb = atn_pool.tile([K, C], bf16, tag="attn")
                    nc.vector.tensor_copy(attnb, ndb)
                    # state += both chunks; one snapshot per pair
                    nc.tensor.matmul(state_ps, vka[:, 0:K],
                                     vka[:, K:2 * K + 1],
                                     start=False, stop=False,
                                     skip_group_check=True)
                    nc.tensor.matmul(state_ps, vkb[:, 0:K],
                                     vkb[:, K:2 * K + 1],
                                     start=False, stop=(p == NPAIR - 1),
                                     skip_group_check=True)
                    if p + 1 < NPAIR:
                        ablocks(p + 1)
                        ks_i = ks_pool.tile([K, 1 + K], bf16, tag="ks")
                        nc.scalar.copy(ks_i, state_ps)
                        ks_prev = ks_i
                    prevs = [(a, attna, recipa), (b, attnb, recipb)]
                for pr in prevs:
                    outproj(*pr)
                # tail: last chunk's store split across the two HWDGE
                # queues so it doesn't serialize the finish
                nc.sync.dma_start(out=out2[:, 15360:15872],
                                  in_=ostage[:, 15360:15872])
                nc.scalar.dma_start(out=out2[:, 15872:16384],
                                    in_=ostage[:, 15872:16384])

    nc.compile()
    worst = []
    for fn in nc.m.functions:
        for blk in fn.blocks:
            for inst in blk.instructions:
                n = len(inst.sync_info.on_wait) if inst.sync_info else 0
                if n > 1 and type(inst).__name__ == "InstMatmult":
                    worst.append((inst.name, n))
    if worst:
        print(f"WARNING: matmuls with >1 wait after lowering: {worst}")
    return nc


def _prep_inputs(x, Wq, Wk, Wv, Wo):
    import ml_dtypes

    bf16 = ml_dtypes.bfloat16
    wkv = np.concatenate([Wk.T, Wv.T], axis=1)                # [D, 2K]
    wqk = np.concatenate([Wq.T, Wk.T], axis=1)                # [D, 2K]
    mask = np.triu(np.ones((C, C), np.float32))               # keep t <= s
    wcat = np.concatenate(
        [wkv[d * 128:(d + 1) * 128, :] for d in range(NDC)]
        + [wqk[d * 128:(d + 1) * 128, :] for d in range(NDC)]
        + [mask],
        axis=1,
    ).astype(bf16)
    wot = np.ascontiguousarray(Wo.T).astype(bf16)             # [K, D]
    zeros_xp = np.zeros((128, NCHUNK * 1024), dtype=bf16)
    in_maps = []
    for core in range(8):
        b, h = core // 2, core % 2
        xb = x[b].astype(bf16)                                # [S, D]
        # main: [dd, g*4096 + d*512 + t]
        xm = xb[h * L:(h + 1) * L, :]                         # [2048, 1024]
        xm4 = xm.reshape(NGRP, G, NDC, 128).transpose(3, 0, 2, 1)
        xtm2 = np.ascontiguousarray(xm4.reshape(128, NGRP * 4096))
        # prefix: [dd, ci*1024 + d*128 + t]
        if h:
            xp = xb[0:L, :]
            xp4 = xp.reshape(NCHUNK, C, NDC, 128).transpose(3, 0, 2, 1)
            xtp2 = np.ascontiguousarray(xp4.reshape(128, NCHUNK * 1024))
        else:
            xtp2 = zeros_xp
        m = {
            "xtp2": xtp2,
            "xtm2": xtm2,
            "wcat": wcat,
            "wot": wot,
            "selc": np.full((C, 1), float(h) / 16.0, np.float32),
        }
        in_maps.append(m)
    return in_maps


def _run(inputs, trace=False):
    from concourse.bass_utils import run_bass_kernel_spmd

    if "nc" not in _cache:
        _cache["nc"] = _build_nc()
    nc = _cache["nc"]
    in_maps = _prep_inputs(
        np.asarray(inputs["x"], np.float32),
        np.asarray(inputs["Wq"], np.float32),
        np.asarray(inputs["Wk"], np.float32),
        np.asarray(inputs["Wv"], np.float32),
        np.asarray(inputs["Wo"], np.float32),
    )
    res = run_bass_kernel_spmd(nc, in_maps, list(range(8)), trace=trace)
    out = np.empty((B, S, D), np.float32)
    for core in range(8):
        b, h = core // 2, core % 2
        o = res.results[core]["out2"].astype(np.float32)
        # [128 t, ci*1024 + dcol] -> [2048, 1024]
        o = o.reshape(128, NCHUNK, 1024).transpose(1, 0, 2).reshape(L, D)
        out[b, h * L:(h + 1) * L, :] = o
    return out, res


def kernel(**inputs) -> np.ndarray:
    out, _ = _run(inputs, trace=False)
    return out


# revision 5
# speedup vs baseline: 1.2967x; 1.1494x over previous
"""Low-rank linear attention (causal, elu+1 feature map) on 8 trn2 cores.

Sharding: core = 2*b + h  (batch b in 0..3, sequence half h in 0..1).
Each core computes out[b, h*2048:(h+1)*2048, :].  Second-half cores
recompute the running K^T.V state over their 2048-token prefix on device
(prefix V contributions are scaled by sel=h so one SPMD program serves
all 8 cores).

v8 structure (trace-driven: the projection phases run warm and
near-roofline; the attention phase always ends up at 1.2 GHz because
HAM re-throttles at the first lull and never recovers — so shrink the
attention phase instead of fighting the clock):
 - PE warm-up: ~11 dummy N=512 matmuls right after the preamble so HAM
   latches K=8/8 before the first real matmul.
 - x DRAM layouts are consumption-ordered: prefix chunk-major,
   main group-major; output chunk-major with 1MB group stores.
 - main projections: K-major [q;k] group matmuls (N=512) + token-major
   [kf|v] chunk matmuls (N=128, same wkv weights as the prefix); the
   per-chunk masked score block is also computed here (warm, dense)
   and parked in SBUF, so the attention phase doesn't pay for it at
   half clock.
 - attention phase (cold): per chunk only num/den/state matmuls plus a
   row-PACKED output projection: attn is duplicated into partitions
   0:64 and 64:128 and Wo.T halves are stacked the same way, so the
   two N=512 contraction-64 matmuls run concurrently in disjoint
   row-groups of the PE array.  Snapshot copy on scalar; evictions
   split scalar/vector.
 - elu+1 = max(z+1, min(exp(z), 1)); V carries 1/16 (num/den scale).
 - PSUM: phase A state(1)+p1(2)+pp2(2)+scores(2) = 7; phase B
   state(1)+ndden(3)+op(4x [C,512]) = 8 banks.

Shapes (hardcoded): B=4, S=4096, D=1024, K=64.  L = S/2 = 2048 tokens
per core, processed in 16 chunks of C=128.
"""

import numpy as np

B, S, D, K = 4, 4096, 1024, 64
L = S // 2          # tokens per core (main), also prefix length
C = 128             # chunk (tokens)
G = 512             # token group for K-major projection matmuls
NCHUNK = L // C     # 16
NGRP = L // G       # 4
NDC = D // 128      # 8 contraction chunks
VS = 1.0 / 16.0     # V scale folded into num and den

_cache = {}


def _build_nc():
    import concourse.bacc as bacc
    import concourse.tile as tile
    from concourse import mybir

    f32 = mybir.dt.float32
    bf16 = mybir.dt.bfloat16
    AF = mybir.ActivationFunctionType
    Alu = mybir.AluOpType

    nc = bacc.Bacc()

    # chunk-major prefix x: [dd, ci*1024 + d*128 + t]
    xtp2 = nc.declare_dram_parameter("xtp2", [128, NCHUNK * 1024], bf16,
                                     isOutput=False)
    # group-major main x: [dd, g*4096 + d*512 + t]
    xtm2 = nc.declare_dram_parameter("xtm2", [128, NGRP * 4096], bf16,
                                     isOutput=False)
    # wcat cols: [wkv 8*128 | wqk 8*128 | mask 128]
    WKV0, WQK0 = 0, NDC * 128
    MSK0 = 2 * NDC * 128
    WCOLS = MSK0 + C
    wcat = nc.declare_dram_parameter("wcat", [128, WCOLS], bf16,
                                     isOutput=False)
    # Wo.T stacked: rows 0:64 = cols 0:512, rows 64:128 = cols 512:1024
    wot2 = nc.declare_dram_parameter("wot2", [128, 512], bf16,
                                     isOutput=False)
    selc = nc.declare_dram_parameter("selc", [C, 1], f32, isOutput=False)
    # chunk-major output: [t, ci*1024 + dcol]
    out2 = nc.declare_dram_parameter("out2", [128, NCHUNK * 1024], bf16,
                                     isOutput=True)

    with tile.TileContext(nc) as tc:
        with (
            tc.tile_pool(name="consts", bufs=1) as consts,
            tc.tile_pool(name="xp", bufs=1) as xp_pool,
            tc.tile_pool(name="xg", bufs=1) as xg_pool,
            tc.tile_pool(name="ost", bufs=1) as ost_pool,
            tc.tile_pool(name="small", bufs=6) as small,
            tc.tile_pool(name="vko", bufs=NCHUNK + 1) as vko_pool,
            tc.tile_pool(name="pvko", bufs=NCHUNK + 1) as pvko_pool,
            tc.tile_pool(name="qk", bufs=NCHUNK + 1) as qk_pool,
            tc.tile_pool(name="atn", bufs=NCHUNK + 2) as atn_pool,
            tc.tile_pool(name="atx", bufs=3) as atx_pool,
            tc.tile_pool(name="ks", bufs=4) as ks_pool,
            tc.tile_pool(name="tmp", bufs=4) as tmp_pool,
            tc.tile_pool(name="ptmp", bufs=4) as ptmp_pool,
            tc.tile_pool(name="state_pool", bufs=1, space="PSUM") as state_pool,
        ):
            # ---- weights/consts on the scalar HWDGE queue (selc + wkv
            # first: the first prefix matmul + evict need them) ----
            selc_sb = consts.tile([C, 1], f32, tag="selc")
            nc.scalar.dma_start(out=selc_sb, in_=selc[:, :])
            wcat_sb = consts.tile([128, WCOLS], bf16, tag="wcat")
            nc.scalar.dma_start(out=wcat_sb[:, WKV0:WKV0 + NDC * 128],
                                in_=wcat[:, WKV0:WKV0 + NDC * 128])
            nc.scalar.dma_start(out=wcat_sb[:, WQK0:WCOLS],
                                in_=wcat[:, WQK0:WCOLS])
            wot_sb = consts.tile([128, 512], bf16, tag="wot")
            nc.scalar.dma_start(out=wot_sb, in_=wot2[:, :])
            wkv_sb = [wcat_sb[:, WKV0 + d * 128:WKV0 + (d + 1) * 128]
                      for d in range(NDC)]
            wqk_sb = [wcat_sb[:, WQK0 + d * 128:WQK0 + (d + 1) * 128]
                      for d in range(NDC)]
            mask_sb = wcat_sb[:, MSK0:MSK0 + C]

            # ---- x transfers on the sync HWDGE queue ----
            xpc = [xp_pool.tile([128, 1024], bf16, name=f"xpc{ci}",
                                tag=f"xpc{ci}") for ci in range(NCHUNK)]
            xg = [xg_pool.tile([128, 4096], bf16, name=f"xg{g}",
                               tag=f"xg{g}") for g in range(NGRP)]
            for ci in range(NCHUNK):
                nc.sync.dma_start(out=xpc[ci],
                                  in_=xtp2[:, ci * 1024:(ci + 1) * 1024])
            for g in range(NGRP):
                for half in range(2):
                    lo = g * 4096 + half * 2048
                    nc.sync.dma_start(
                        out=xg[g][:, half * 2048:(half + 1) * 2048],
                        in_=xtm2[:, lo:lo + 2048])

            # den-sum column: carries the same 1/16 scale as V's columns
            onec_sb = consts.tile([C, 1], bf16, tag="onec")
            nc.vector.memset(onec_sb, VS)
            # persistent output staging (chunk-major, matches out2)
            ostage = ost_pool.tile([128, NCHUNK * 1024], bf16, tag="ostage")

            # running state [K, K+1]: cols 0:K = S'[k,m] (1/16-scaled),
            # col K = k_sum/16.
            state_ps = state_pool.tile([K, 1 + K], f32)

            # =============== PE warm-up: dummy matmuls on zeros =======
            wz = consts.tile([128, G], bf16, tag="wz")
            nc.vector.memset(wz, 0.0)

            # =============== PREFIX: token-major [K|V], state sum ======
            pvkos = []
            with tc.tile_pool(name="pp_ps", bufs=4, space="PSUM") as pp_pool:
                with tc.tile_pool(name="warm_ps", bufs=1,
                                  space="PSUM") as warm_pool:
                    warm_ps = warm_pool.tile([128, G], f32)
                    for i in range(11):
                        nc.tensor.matmul(warm_ps, wz[:, 0:128], wz,
                                         start=True, stop=True)
                for ci in range(NCHUNK):
                    pp = pp_pool.tile([C, 2 * K], f32, tag="pp")
                    for d in range(NDC):
                        nc.tensor.matmul(pp,
                                         xpc[ci][:, d * 128:(d + 1) * 128],
                                         wkv_sb[d],
                                         start=(d == 0), stop=(d == NDC - 1))
                    eu = ptmp_pool.tile([C, K], f32, tag="eu")
                    nc.scalar.activation(eu, pp[:, 0:K], AF.Exp)
                    em = ptmp_pool.tile([C, K], f32, tag="em")
                    nc.vector.tensor_scalar_min(em, eu, 1.0)
                    pvko = pvko_pool.tile([C, 2 * K + 1], bf16, tag="pvko")
                    nc.vector.scalar_tensor_tensor(
                        pvko[:, 0:K], pp[:, 0:K], 1.0, em,
                        op0=Alu.add, op1=Alu.max)
                    # V and ones column scaled by sel/16 (zero on h=0)
                    nc.scalar.activation(pvko[:, K:2 * K], pp[:, K:2 * K],
                                         AF.Copy, scale=selc_sb)
                    nc.gpsimd.tensor_copy(pvko[:, 2 * K:2 * K + 1], selc_sb)
                    pvkos.append(pvko)
                # state updates after all projections: PE never stalls
                for ci in range(NCHUNK):
                    pvko = pvkos[ci]
                    nc.tensor.matmul(state_ps, pvko[:, 0:K],
                                     pvko[:, K:2 * K + 1],
                                     start=(ci == 0), stop=False,
                                     skip_group_check=True)

            # prefix state snapshot (ks for chunk 0)
            ks_init = ks_pool.tile([K, 1 + K], bf16, tag="ks")
            nc.scalar.copy(ks_init, state_ps)

            # =============== MAIN projections + score blocks ==========
            qTs, vkos, atms = {}, {}, {}
            with (
                tc.tile_pool(name="p1_ps", bufs=2, space="PSUM") as p1_pool,
                tc.tile_pool(name="pp2_ps", bufs=2, space="PSUM") as pp2_pool,
                tc.tile_pool(name="sc_ps", bufs=2, space="PSUM") as sc_pool,
            ):
                for g in range(NGRP):
                    p1g = p1_pool.tile([2 * K, G], f32, tag="p1")
                    for d in range(NDC):
                        nc.tensor.matmul(p1g, wqk_sb[d],
                                         xg[g][:, d * 512:(d + 1) * 512],
                                         start=(d == 0), stop=(d == NDC - 1))
                    eg = tmp_pool.tile([2 * K, G], f32, tag="eg", bufs=2)
                    nc.scalar.activation(eg, p1g, AF.Exp)
                    eg2 = tmp_pool.tile([2 * K, G], f32, tag="eg2", bufs=2)
                    nc.vector.tensor_scalar_min(eg2, eg, 1.0)
                    for c4 in range(4):
                        ci = g * 4 + c4
                        sl = slice(c4 * C, (c4 + 1) * C)
                        kT = qk_pool.tile([K, C], bf16, tag="kT")
                        nc.vector.scalar_tensor_tensor(
                            kT, p1g[K:2 * K, sl], 1.0, eg2[K:2 * K, sl],
                            op0=Alu.add, op1=Alu.max)
                        qT = qk_pool.tile([K, C], bf16, tag="qT")
                        nc.vector.scalar_tensor_tensor(
                            qT, p1g[0:K, sl], 1.0, eg2[0:K, sl],
                            op0=Alu.add, op1=Alu.max)
                        qTs[ci] = qT
                        # token-major [kf|v|1] (same wkv as the prefix)
                        pp2 = pp2_pool.tile([C, 2 * K], f32, tag="pp2")
                        for d in range(NDC):
                            lo = d * 512 + c4 * 128
                            nc.tensor.matmul(pp2, xg[g][:, lo:lo + 128],
                                             wkv_sb[d],
                                             start=(d == 0),
                                             stop=(d == NDC - 1))
                        eu2 = ptmp_pool.tile([C, K], f32, tag="eu2")
                        nc.scalar.activation(eu2, pp2[:, 0:K], AF.Exp)
                        em2 = ptmp_pool.tile([C, K], f32, tag="em2")
                        nc.vector.tensor_scalar_min(em2, eu2, 1.0)
                        vko = vko_pool.tile([C, 2 * K + 1], bf16, tag="vko")
                        nc.vector.scalar_tensor_tensor(
                            vko[:, 0:K], pp2[:, 0:K], 1.0, em2,
                            op0=Alu.add, op1=Alu.max)
                        nc.scalar.activation(vko[:, K:2 * K],
                                             pp2[:, K:2 * K],
                                             AF.Copy, scale=VS)
                        nc.gpsimd.memset(vko[:, 2 * K:2 * K + 1], VS)
                        vkos[ci] = vko
                        # masked score block now, while the PE is warm
                        at = sc_pool.tile([C, C], f32, tag="sc")
                        nc.tensor.matmul(at, kT, qT, start=True, stop=True)
                        atm = atn_pool.tile([C, C], bf16, tag="atm")
                        nc.vector.tensor_tensor(atm, at, mask_sb, Alu.mult)
                        atms[ci] = atm

            # =============== PHASE B: attention =======================
            with (
                tc.tile_pool(name="nd_ps", bufs=3, space="PSUM") as nd_pool,
                tc.tile_pool(name="op_ps", bufs=4, space="PSUM") as op_pool,
            ):
                def outproj(ci, attnx, recip):
                    op1 = op_pool.tile([C, 512], f32, tag="op")
                    op2 = op_pool.tile([C, 512], f32, tag="op")
                    # row-packed: the two contraction-64 matmuls occupy
                    # disjoint row-groups of the array and run
                    # concurrently
                    nc.tensor.matmul(op1, attnx[0:K, :], wot_sb[0:K, :],
                                     start=True, stop=True)
                    nc.tensor.matmul(op2, attnx[K:2 * K, :],
                                     wot_sb[K:2 * K, :],
                                     start=True, stop=True)
                    lo = ci * 1024
                    nc.scalar.activation(ostage[:, lo:lo + 512], op1,
                                         AF.Copy, scale=recip)
                    nc.vector.tensor_scalar_mul(
                        ostage[:, lo + 512:lo + 1024], op2, recip)
                    if ci == 3:
                        nc.sync.dma_start(out=out2[:, 0:4096],
                                          in_=ostage[:, 0:4096])
                    elif ci == 7:
                        nc.sync.dma_start(out=out2[:, 4096:8192],
                                          in_=ostage[:, 4096:8192])
                    elif ci == 11:
                        nc.sync.dma_start(out=out2[:, 8192:12288],
                                          in_=ostage[:, 8192:12288])
                    elif ci == 14:
                        nc.sync.dma_start(out=out2[:, 12288:15360],
                                          in_=ostage[:, 12288:15360])

                ks_prev = ks_init
                prev = None  # (ci, attnx, recip) pending output projection
                for ci in range(NCHUNK):
                    qT, vko, atm = qTs[ci], vkos[ci], atms.pop(ci)
                    if prev is not None:
                        outproj(*prev)
                    # [num'^T] = v'^T A + S'^T q
                    nd = nd_pool.tile([K, C], f32, tag="nd")
                    nc.tensor.matmul(nd, vko[:, K:2 * K], atm,
                                     start=True, stop=False)
                    nc.tensor.matmul(nd, ks_prev[:, 0:K], qT,
                                     start=False, stop=True)
                    # den' transposed directly on the PE: [C,1]; recip
                    # reads PSUM — no vector hop inside the PE chain
                    den = nd_pool.tile([C, 1], f32, tag="nd")
                    nc.tensor.matmul(den, atm, onec_sb, start=True,
                                     stop=False)
                    nc.tensor.matmul(den, qT, ks_prev[:, K:K + 1],
                                     start=False, stop=True)
                    # state update + snapshot (snapshot on scalar: the
                    # vector queue stays off the serial chain)
                    nc.tensor.matmul(state_ps, vko[:, 0:K],
                                     vko[:, K:2 * K + 1],
                                     start=False, stop=(ci == NCHUNK - 1),
                                     skip_group_check=True)
                    if ci < NCHUNK - 1:
                        ks_i = ks_pool.tile([K, 1 + K], bf16, tag="ks")
                        nc.scalar.copy(ks_i, state_ps)
                        ks_prev = ks_i
                    recip = small.tile([C, 1], f32, tag="recip")
                    nc.vector.reciprocal(recip, den)
                    # attn duplicated into both row halves for the
                    # packed output projection
                    attnx = atx_pool.tile([2 * K, C], bf16, tag="attnx")
                    nc.vector.tensor_copy(attnx[0:K, :], nd)
                    nc.vector.tensor_copy(attnx[K:2 * K, :], nd)
                    prev = (ci, attnx, recip)
                outproj(*prev)
                # tail: last chunk's store split across the two HWDGE
                # queues so it doesn't serialize the finish
                nc.sync.dma_start(out=out2[:, 15360:15872],
                                  in_=ostage[:, 15360:15872])
                nc.scalar.dma_start(out=out2[:, 15872:16384],
                                    in_=ostage[:, 15872:16384])

    nc.compile()
    worst = []
    for fn in nc.m.functions:
        for blk in fn.blocks:
            for inst in blk.instructions:
                n = len(inst.sync_info.on_wait) if inst.sync_info else 0
                if n > 1 and type(inst).__name__ == "InstMatmult":
                    worst.append((inst.name, n))
    if worst:
        print(f"WARNING: matmuls with >1 wait after lowering: {worst}")
    return nc


def _prep_inputs(x, Wq, Wk, Wv, Wo):
    import ml_dtypes

    bf16 = ml_dtypes.bfloat16
    wkv = np.concatenate([Wk.T, Wv.T], axis=1)                # [D, 2K]
    wqk = np.concatenate([Wq.T, Wk.T], axis=1)                # [D, 2K]
    mask = np.triu(np.ones((C, C), np.float32))               # keep t <= s
    wcat = np.concatenate(
        [wkv[d * 128:(d + 1) * 128, :] for d in range(NDC)]
        + [wqk[d * 128:(d + 1) * 128, :] for d in range(NDC)]
        + [mask],
        axis=1,
    ).astype(bf16)
    wotT = Wo.T                                               # [K, D]
    wot2 = np.concatenate([wotT[:, 0:512], wotT[:, 512:1024]],
                          axis=0).astype(bf16)                # [128, 512]
    zeros_xp = np.zeros((128, NCHUNK * 1024), dtype=bf16)
    in_maps = []
    for core in range(8):
        b, h = core // 2, core % 2
        xb = x[b].astype(bf16)                                # [S, D]
        # main: [dd, g*4096 + d*512 + t]
        xm = xb[h * L:(h + 1) * L, :]                         # [2048, 1024]
        xm4 = xm.reshape(NGRP, G, NDC, 128).transpose(3, 0, 2, 1)
        xtm2 = np.ascontiguousarray(xm4.reshape(128, NGRP * 4096))
        # prefix: [dd, ci*1024 + d*128 + t]
        if h:
            xp = xb[0:L, :]
            xp4 = xp.reshape(NCHUNK, C, NDC, 128).transpose(3, 0, 2, 1)
            xtp2 = np.ascontiguousarray(xp4.reshape(128, NCHUNK * 1024))
        else:
            xtp2 = zeros_xp
        m = {
            "xtp2": xtp2,
            "xtm2": xtm2,
            "wcat": wcat,
            "wot2": wot2,
            "selc": np.full((C, 1), float(h) / 16.0, np.float32),
        }
        in_maps.append(m)
    return in_maps


def _run(inputs, trace=False):
    from concourse.bass_utils import run_bass_kernel_spmd

    if "nc" not in _cache:
        _cache["nc"] = _build_nc()
    nc = _cache["nc"]
    in_maps = _prep_inputs(
        np.asarray(inputs["x"], np.float32),
        np.asarray(inputs["Wq"], np.float32),
        np.asarray(inputs["Wk"], np.float32),
        np.asarray(inputs["Wv"], np.float32),
        np.asarray(inputs["Wo"], np.float32),
    )
    res = run_bass_kernel_spmd(nc, in_maps, list(range(8)), trace=trace)
    out = np.empty((B, S, D), np.float32)
    for core in range(8):
        b, h = core // 2, core % 2
        o = res.results[core]["out2"].astype(np.float32)
        # [128 t, ci*1024 + dcol] -> [2048, 1024]
        o = o.reshape(128, NCHUNK, 1024).transpose(1, 0, 2).reshape(L, D)
        out[b, h * L:(h + 1) * L, :] = o
    return out, res


def kernel(**inputs) -> np.ndarray:
    out, _ = _run(inputs, trace=False)
    return out
